# revision 22
# baseline (speedup 1.0000x reference)
"""GraphSAGE-mean (DivFeatConv) forward on 8 TRN2 NeuronCores.

out = relu(feat @ W_self.T + b_self + segmean(feat[src], dst) @ W_neigh.T + b_neigh)

Strategy (SPMD, one program on 8 cores):
  - Nodes are relabeled by in-degree (descending) and dealt round-robin into
    128-node dst tiles so that every tile holds similar-degree nodes; tiles
    are dealt round-robin to cores so the per-tile block count K_j (a shared
    template constant = max over cores) is tight (~4% padding).
  - The host stages, per core, a "message image" in DRAM laid out exactly as
    the SBUF tile the kernel wants: partition p, block (B0[j]+k) holds the
    fp8 features of the k-th in-edge src of the j-th tile's p-th node (zero
    rows pad).  Slot POSITION encodes the dst node, so the scatter-sum is a
    matmul against a constant identity matrix and the "gather" is a plain
    sequential strided DMA - no SWDGE descriptor generation, no per-block
    selection matrices.
  - Aggregation: per dst tile, K_j blocks are summed in PSUM with fp8
    DoubleRow matmuls (two 128-slot blocks per instruction, 0.5 cyc/row):
    ps1[d, n] += msg_blk[slot, d]^T @ I[slot, n].
  - VectorE multiplies by 1/deg (bf16), TensorE applies W_self/W_neigh per
    512-column supertile, ScalarE fuses bias+ReLU, bf16 result DMAs out; the
    host casts to f32 and scatters rows back through the relabeling.
  - Message DMAs rotate over the sync/scalar/vector HWDGE queues and are
    sized ~65 blocks (~1MB) by greedy supertile balancing; stage 2 for
    supertile s is emitted after the aggregation matmuls of supertile s+1 so
    TensorE never waits on VectorE.

All template constants (K_j schedule) are maxima over cores, so one SPMD
program serves all 8 cores with per-core tables.
"""

import numpy as np
import ml_dtypes

import concourse.bacc as bacc
import concourse.bass as bass
import concourse.mybir as mybir
import concourse.tile as tile
from concourse.bass_utils import run_bass_kernel_spmd

BF16 = ml_dtypes.bfloat16
FP8 = ml_dtypes.float8_e4m3
P = 128
NCORES = 8
ST = 4  # dst tiles per supertile (ps2 = [128, ST*128] f32 = one PSUM bank)
DELAY_CHUNKS = 0  # message chunks landed before the PE is released

# stash of the last compiled/run state so test harnesses can re-run with
# tracing enabled
LAST = {}


def _make_plan(src, dst, n_nodes):
    """Host-side relabeling / edge partitioning. Returns the shared template
    plus per-core node/edge layout."""
    N = n_nodes
    E = len(dst)
    deg = np.bincount(dst, minlength=N)
    order = np.argsort(-deg, kind="stable").astype(np.int64)
    NTG = -(-N // P)          # global 128-node tiles
    NT = -(-NTG // NCORES)    # local tiles per core
    NPC = NT * P              # output columns per core

    # core m local tile j <- global tile NCORES*j + m (degree-sorted)
    nodes = np.full((NCORES, NT, P), -1, np.int64)
    for j in range(NT):
        for m in range(NCORES):
            g = NCORES * j + m
            if g >= NTG:
                continue
            ids = order[P * g : P * g + P]
            nodes[m, j, : len(ids)] = ids

    degw = np.where(nodes >= 0, deg[np.clip(nodes, 0, None)], 0)
    KT = degw.max(axis=2).max(axis=0)  # [NT] blocks per tile (template)

    # greedy-balance tiles into NT/ST supertiles of exactly ST tiles each,
    # then flatten back into a processing order; columns/blocks follow the
    # processing order so DMA chunks and ps2 groups stay contiguous.
    NS = NT // ST
    bins = [[] for _ in range(NS)]
    loads = np.zeros(NS, np.int64)
    for t in np.argsort(-KT, kind="stable"):
        open_bins = [b for b in range(NS) if len(bins[b]) < ST]
        b = min(open_bins, key=lambda x: (loads[x], x))
        bins[b].append(int(t))
        loads[b] += KT[t]
    proc = np.array([t for b in bins for t in sorted(b)], np.int64)

    nodes = nodes[:, proc, :]          # [NC, NT, P] in processing order
    KTp = KT[proc]
    BLK0 = np.zeros(NT + 1, np.int64)
    np.cumsum(KTp, out=BLK0[1:])
    TOTBLK = int(BLK0[-1])

    # per-node placement maps (for vectorized edge binning)
    node_core = np.full(N, -1, np.int64)
    node_col = np.zeros(N, np.int64)    # column within core [0, NPC)
    node_blk0 = np.zeros(N, np.int64)   # block offset of its tile
    cols = np.arange(NT * P, dtype=np.int64)
    blk0_of_col = np.repeat(BLK0[:-1], P)
    for m in range(NCORES):
        nz = nodes[m].reshape(-1)
        v = nz >= 0
        node_core[nz[v]] = m
        node_col[nz[v]] = cols[v]
        node_blk0[nz[v]] = blk0_of_col[v]

    # CSR by dst
    eorder = np.argsort(dst, kind="stable")
    dst_s = dst[eorder]
    src_s = src[eorder]
    indptr = np.zeros(N + 1, np.int64)
    np.cumsum(deg, out=indptr[1:])
    k_e = np.arange(E, dtype=np.int64) - indptr[dst_s]  # rank within dst

    recip = (1.0 / np.maximum(deg, 1)).astype(np.float32)

    return dict(
        N=N,
        NT=NT,
        NPC=NPC,
        KT=KTp,
        BLK0=BLK0,
        TOTBLK=TOTBLK,
        nodes=nodes,
        node_core=node_core,
        node_col=node_col,
        node_blk0=node_blk0,
        dst_s=dst_s,
        src_s=src_s,
        k_e=k_e,
        recip=recip,
    )


def _make_inputs(plan, feat, W_self, b_self, W_neigh, b_neigh):
    N = plan["N"]
    NPC = plan["NPC"]
    TOTBLK = plan["TOTBLK"]
    nodes = plan["nodes"]

    feat8x = np.vstack([feat.astype(FP8), np.zeros((1, P), FP8)])
    featbx = np.vstack([feat.astype(BF16), np.zeros((1, P), BF16)])
    recipx = np.append(plan["recip"], 0.0).astype(BF16)

    wsT = np.ascontiguousarray(np.asarray(W_self, np.float32).T).astype(BF16)
    wnT = np.ascontiguousarray(np.asarray(W_neigh, np.float32).T).astype(BF16)
    bias = (
        (np.asarray(b_self, np.float32) + np.asarray(b_neigh, np.float32))
        .astype(np.float32)
        .reshape(P, 1)
    )
    ident2 = np.zeros((P, 2 * P), FP8)
    ident2[np.arange(P), np.arange(P)] = 1.0
    ident2[np.arange(P), P + np.arange(P)] = 1.0

    m_e = plan["node_core"][plan["dst_s"]]
    flat_e = (plan["node_blk0"][plan["dst_s"]] + plan["k_e"]) * P + (
        plan["node_col"][plan["dst_s"]] % P
    )

    in_maps = []
    for m in range(NCORES):
        grid = np.full(TOTBLK * P, N, np.int64)  # default -> zero row
        sel = m_e == m
        grid[flat_e[sel]] = plan["src_s"][sel]
        msgs = feat8x[grid.reshape(TOTBLK, P)]  # [TOTBLK, P, 128]
        msgs = np.ascontiguousarray(
            msgs.transpose(1, 0, 2).reshape(P, TOTBLK * P)
        )
        nz = nodes[m].reshape(-1)
        nidx = np.where(nz >= 0, nz, N)
        ftT = np.ascontiguousarray(featbx[nidx].T)
        rb = np.ascontiguousarray(np.broadcast_to(recipx[nidx], (P, NPC)))
        in_maps.append(
            dict(
                msgs=msgs,
                featT=ftT,
                recipb=rb,
                wsT=wsT,
                wnT=wnT,
                bias=bias,
                ident2=ident2,
            )
        )
    return in_maps


def _build(plan):
    NT = plan["NT"]
    NPC = plan["NPC"]
    KT = plan["KT"]
    BLK0 = plan["BLK0"]
    TOTBLK = plan["TOTBLK"]

    f32 = mybir.dt.float32
    bf16 = mybir.dt.bfloat16
    fp8 = mybir.dt.float8e4
    NS = NT // ST

    nc = bacc.Bacc(
        "TRN2",
        target_bir_lowering=False,
        debug=False,
        num_devices=NCORES,
    )

    msgs_t = nc.dram_tensor("msgs", [P, TOTBLK * P], fp8, kind="ExternalInput")
    ftT_t = nc.dram_tensor("featT", [P, NPC], bf16, kind="ExternalInput")
    rb_t = nc.dram_tensor("recipb", [P, NPC], bf16, kind="ExternalInput")
    wsT_t = nc.dram_tensor("wsT", [P, P], bf16, kind="ExternalInput")
    wnT_t = nc.dram_tensor("wnT", [P, P], bf16, kind="ExternalInput")
    bias_t = nc.dram_tensor("bias", [P, 1], f32, kind="ExternalInput")
    id_t = nc.dram_tensor("ident2", [P, 2 * P], fp8, kind="ExternalInput")
    out_t = nc.dram_tensor("out", [P, NPC], bf16, kind="ExternalOutput")

    with tile.TileContext(nc) as tc:
        with (
            tc.tile_pool(name="const", bufs=1) as cpool,
            tc.tile_pool(name="hbuf", bufs=4 * ST) as hpool,
            tc.tile_pool(name="ps1", bufs=6, space="PSUM") as p1pool,
            tc.tile_pool(name="ps2", bufs=2, space="PSUM") as p2pool,
        ):
            id_sb = cpool.tile([P, 2 * P], fp8, tag="ident2")
            wsT_sb = cpool.tile([P, P], bf16, tag="ws")
            wnT_sb = cpool.tile([P, P], bf16, tag="wn")
            bias_sb = cpool.tile([P, 1], f32, tag="bias")
            rb_sb = cpool.tile([P, NPC], bf16, tag="rb")
            ftT_sb = cpool.tile([P, NPC], bf16, tag="ftT")
            out_sb = cpool.tile([P, NPC], bf16, tag="out")
            msgs_sb = cpool.tile([P, TOTBLK * P], fp8, tag="msgs")

            # A priority-ordered DMA stream ping-ponged over the two HWDGE
            # queues (so one chunk's transfer overlaps the next chunk's
            # descriptor generation): the first (small) message chunk
            # unblocks TensorE ASAP; weights / recip / featT are slotted in
            # just before their first consumer would need them so they never
            # delay the message stream.
            qs = [nc.sync, nc.scalar]
            qi = [0]

            def issue(out_ap, in_ap):
                qs[qi[0] % len(qs)].dma_start(out_ap, in_ap)
                qi[0] += 1

            # message chunk boundaries: small taper-in (fast PE start), 48
            # blocks steady state, taper-out (small tail)
            head = [8, 16, 32]
            tail = [24, 16, 8]
            mid = TOTBLK - sum(head) - sum(tail)
            sizes = head + [48] * (mid // 48) + ([mid % 48] if mid % 48 else []) + tail
            bounds = np.concatenate([[0], np.cumsum(sizes)])
            assert bounds[-1] == TOTBLK

            # supertile end-blocks: rb slice s is first read by hb(s) (right
            # after agg(s)); ftT slice s is first read by stage2(s), which the
            # PE reaches right after agg(s+1) (pending depth 1)
            EB = [int(BLK0[(s + 1) * ST]) for s in range(NS)]
            done_s = 0
            W = ST * P
            for ci in range(len(bounds) - 1):
                b0, b1 = int(bounds[ci]), int(bounds[ci + 1])
                issue(msgs_sb[:, b0 * P : b1 * P], msgs_t.ap()[:, b0 * P : b1 * P])
                if ci == DELAY_CHUNKS:
                    issue(id_sb[:], id_t.ap()[:])
                while done_s < NS and EB[done_s] <= b1:
                    s = done_s
                    if s == 0:
                        issue(wsT_sb[:], wsT_t.ap()[:])
                        issue(wnT_sb[:], wnT_t.ap()[:])
                        issue(bias_sb[:], bias_t.ap()[:])
                    issue(
                        rb_sb[:, s * W : (s + 1) * W], rb_t.ap()[:, s * W : (s + 1) * W]
                    )
                    if s >= 1:
                        f = s - 1
                        issue(
                            ftT_sb[:, f * W : (f + 1) * W],
                            ftT_t.ap()[:, f * W : (f + 1) * W],
                        )
                    done_s += 1
            for f in range(max(0, done_s - 1), NS):
                issue(
                    ftT_sb[:, f * W : (f + 1) * W], ftT_t.ap()[:, f * W : (f + 1) * W]
                )

            id3 = id_sb[:].rearrange("p (i e) -> p i e", e=P)
            msg3 = msgs_sb[:].rearrange("p (b e) -> p b e", e=P)
            DR = mybir.MatmulPerfMode.DoubleRow

            def emit_finish(fi):
                c0 = fi["s"] * ST * P
                ps2 = p2pool.tile([P, ST * P], f32, tag="ps2")
                for i, t in enumerate(fi["tiles"]):
                    sl = ps2[:, i * P : (i + 1) * P]
                    hb = fi["hbs"][i]
                    nc.tensor.matmul(
                        sl,
                        lhsT=wsT_sb[:],
                        rhs=ftT_sb[:, (c0 + i * P) : (c0 + (i + 1) * P)],
                        start=True,
                        stop=hb is None,
                    )
                    if hb is not None:
                        nc.tensor.matmul(
                            sl, lhsT=wnT_sb[:], rhs=hb[:], start=False, stop=True
                        )
                nc.scalar.activation(
                    out_sb[:, c0 : c0 + ST * P],
                    ps2[:],
                    mybir.ActivationFunctionType.Relu,
                    bias=bias_sb[:, 0:1],
                )
                nc.gpsimd.dma_start(
                    out_t.ap()[:, c0 : c0 + ST * P], out_sb[:, c0 : c0 + ST * P]
                )

            pending = []
            for s in range(NS):
                tiles = list(range(s * ST, (s + 1) * ST))
                cb = int(BLK0[tiles[-1] + 1]) - int(BLK0[tiles[0]])
                hbs = []
                for i, t in enumerate(tiles):
                    k = int(KT[t])
                    if k == 0:
                        hbs.append(None)
                        continue
                    b0 = int(BLK0[t])
                    ps1t = p1pool.tile([P, P], f32, tag="ps1")
                    ps1 = ps1t[:]
                    npair = k // 2
                    for kk in range(npair):
                        nc.tensor.matmul(
                            ps1,
                            lhsT=msg3[:, b0 + 2 * kk : b0 + 2 * kk + 2, :],
                            rhs=id3,
                            start=(kk == 0),
                            stop=(kk == npair - 1 and k % 2 == 0),
                            perf_mode=DR,
                        )
                    if k % 2:
                        nc.tensor.matmul(
                            ps1,
                            lhsT=msg3[:, b0 + k - 1, :],
                            rhs=id3[:, 0, :],
                            start=(k == 1),
                            stop=True,
                        )
                    hb = hpool.tile([P, P], bf16, tag="hbuf")
                    nc.vector.tensor_tensor(
                        out=hb[:],
                        in0=ps1,
                        in1=rb_sb[:, t * P : (t + 1) * P],
                        op=mybir.AluOpType.mult,
                    )
                    hbs.append(hb)
                fi = dict(s=s, tiles=tiles, hbs=hbs)
                pending.append(fi)
                if len(pending) > 1:
                    emit_finish(pending.pop(0))
            while pending:
                emit_finish(pending.pop(0))

    nc.compile()
    return nc


def kernel(feat, src, dst, W_self, b_self, W_neigh, b_neigh):
    feat = np.asarray(feat, np.float32)
    src = np.asarray(src, np.int64)
    dst = np.asarray(dst, np.int64)
    N, D = feat.shape
    assert D == P

    plan = _make_plan(src, dst, N)
    in_maps = _make_inputs(plan, feat, W_self, b_self, W_neigh, b_neigh)

    key = (N, D, plan["TOTBLK"], plan["KT"].tobytes())
    if LAST.get("key") != key:
        nc = _build(plan)
        LAST.update(key=key, nc=nc)
    nc = LAST["nc"]
    LAST["in_maps"] = in_maps

    res = run_bass_kernel_spmd(nc, in_maps, core_ids=list(range(NCORES)))
    out = np.zeros((N, P), np.float32)
    for m in range(NCORES):
        o = np.asarray(res.results[m]["out"]).astype(np.float32)  # [P, NPC]
        nz = plan["nodes"][m].reshape(-1)
        v = nz >= 0
        out[nz[v]] = o[:, v].T
    return out


# revision 25
# speedup vs baseline: 1.0882x; 1.0882x over previous
"""GraphSAGE-mean (DivFeatConv) forward on 8 TRN2 NeuronCores.

out = relu(feat @ W_self.T + b_self + segmean(feat[src], dst) @ W_neigh.T + b_neigh)

Strategy (SPMD, one program on 8 cores):
  - Nodes are relabeled by in-degree (descending) and dealt round-robin into
    128-node dst tiles so that every tile holds similar-degree nodes; tiles
    are dealt round-robin to cores so the per-tile block count K_j (a shared
    template constant = max over cores) is tight (~4% padding).
  - The host stages, per core, a "message image" in DRAM laid out exactly as
    the SBUF tile the kernel wants: partition p, block (B0[j]+k) holds the
    fp8 features of the k-th in-edge src of the j-th tile's p-th node (zero
    rows pad).  Slot POSITION encodes the dst node, so the scatter-sum is a
    matmul against a constant identity matrix and the "gather" is a plain
    sequential strided DMA - no SWDGE descriptor generation, no per-block
    selection matrices.
  - Aggregation: per dst tile, K_j blocks are summed in PSUM with fp8
    DoubleRow matmuls (two 128-slot blocks per instruction, 0.5 cyc/row):
    ps1[d, n] += msg_blk[slot, d]^T @ I[slot, n].
  - VectorE multiplies by 1/deg (bf16), TensorE applies W_self/W_neigh per
    512-column supertile, ScalarE fuses bias+ReLU, bf16 result DMAs out; the
    host casts to f32 and scatters rows back through the relabeling.
  - Message DMAs rotate over the sync/scalar/vector HWDGE queues and are
    sized ~65 blocks (~1MB) by greedy supertile balancing; stage 2 for
    supertile s is emitted after the aggregation matmuls of supertile s+1 so
    TensorE never waits on VectorE.

All template constants (K_j schedule) are maxima over cores, so one SPMD
program serves all 8 cores with per-core tables.
"""

import numpy as np
import ml_dtypes

import concourse.bacc as bacc
import concourse.bass as bass
import concourse.mybir as mybir
import concourse.tile as tile
from concourse.bass_utils import run_bass_kernel_spmd

BF16 = ml_dtypes.bfloat16
FP8 = ml_dtypes.float8_e4m3
P = 128
NCORES = 8
ST = 4  # dst tiles per supertile (ps2 = [128, ST*128] f32 = one PSUM bank)
DELAY_CHUNKS = 0  # message chunks landed before the PE is released

# stash of the last compiled/run state so test harnesses can re-run with
# tracing enabled
LAST = {}


def _make_plan(src, dst, n_nodes):
    """Host-side relabeling / edge partitioning. Returns the shared template
    plus per-core node/edge layout."""
    N = n_nodes
    E = len(dst)
    deg = np.bincount(dst, minlength=N)
    order = np.argsort(-deg, kind="stable").astype(np.int64)
    NTG = -(-N // P)          # global 128-node tiles
    NT = -(-NTG // NCORES)    # local tiles per core
    NPC = NT * P              # output columns per core

    # core m local tile j <- global tile NCORES*j + m (degree-sorted)
    nodes = np.full((NCORES, NT, P), -1, np.int64)
    for j in range(NT):
        for m in range(NCORES):
            g = NCORES * j + m
            if g >= NTG:
                continue
            ids = order[P * g : P * g + P]
            nodes[m, j, : len(ids)] = ids

    degw = np.where(nodes >= 0, deg[np.clip(nodes, 0, None)], 0)
    KT = degw.max(axis=2).max(axis=0)  # [NT] blocks per tile (template)

    # greedy-balance tiles into NT/ST supertiles of exactly ST tiles each,
    # then flatten back into a processing order; columns/blocks follow the
    # processing order so DMA chunks and ps2 groups stay contiguous.
    NS = NT // ST
    bins = [[] for _ in range(NS)]
    loads = np.zeros(NS, np.int64)
    for t in np.argsort(-KT, kind="stable"):
        open_bins = [b for b in range(NS) if len(bins[b]) < ST]
        b = min(open_bins, key=lambda x: (loads[x], x))
        bins[b].append(int(t))
        loads[b] += KT[t]
    proc = np.array([t for b in bins for t in sorted(b)], np.int64)

    nodes = nodes[:, proc, :]          # [NC, NT, P] in processing order
    KTp = KT[proc]
    BLK0 = np.zeros(NT + 1, np.int64)
    np.cumsum(KTp, out=BLK0[1:])
    TOTBLK = int(BLK0[-1])

    # per-node placement maps (for vectorized edge binning)
    node_core = np.full(N, -1, np.int64)
    node_col = np.zeros(N, np.int64)    # column within core [0, NPC)
    node_blk0 = np.zeros(N, np.int64)   # block offset of its tile
    cols = np.arange(NT * P, dtype=np.int64)
    blk0_of_col = np.repeat(BLK0[:-1], P)
    for m in range(NCORES):
        nz = nodes[m].reshape(-1)
        v = nz >= 0
        node_core[nz[v]] = m
        node_col[nz[v]] = cols[v]
        node_blk0[nz[v]] = blk0_of_col[v]

    # CSR by dst
    eorder = np.argsort(dst, kind="stable")
    dst_s = dst[eorder]
    src_s = src[eorder]
    indptr = np.zeros(N + 1, np.int64)
    np.cumsum(deg, out=indptr[1:])
    k_e = np.arange(E, dtype=np.int64) - indptr[dst_s]  # rank within dst

    recip = (1.0 / np.maximum(deg, 1)).astype(np.float32)

    return dict(
        N=N,
        NT=NT,
        NPC=NPC,
        KT=KTp,
        BLK0=BLK0,
        TOTBLK=TOTBLK,
        nodes=nodes,
        node_core=node_core,
        node_col=node_col,
        node_blk0=node_blk0,
        dst_s=dst_s,
        src_s=src_s,
        k_e=k_e,
        recip=recip,
    )


def _make_inputs(plan, feat, W_self, b_self, W_neigh, b_neigh):
    N = plan["N"]
    NPC = plan["NPC"]
    TOTBLK = plan["TOTBLK"]
    nodes = plan["nodes"]

    feat8x = np.vstack([feat.astype(FP8), np.zeros((1, P), FP8)])
    featbx = np.vstack([feat.astype(BF16), np.zeros((1, P), BF16)])
    recipx = np.append(plan["recip"], 0.0).astype(BF16)

    wsT = np.ascontiguousarray(np.asarray(W_self, np.float32).T).astype(BF16)
    wnT = np.ascontiguousarray(np.asarray(W_neigh, np.float32).T).astype(BF16)
    bias = (
        (np.asarray(b_self, np.float32) + np.asarray(b_neigh, np.float32))
        .astype(np.float32)
        .reshape(P, 1)
    )
    ident2 = np.zeros((P, 2 * P), FP8)
    ident2[np.arange(P), np.arange(P)] = 1.0
    ident2[np.arange(P), P + np.arange(P)] = 1.0

    m_e = plan["node_core"][plan["dst_s"]]
    flat_e = (plan["node_blk0"][plan["dst_s"]] + plan["k_e"]) * P + (
        plan["node_col"][plan["dst_s"]] % P
    )

    in_maps = []
    for m in range(NCORES):
        grid = np.full(TOTBLK * P, N, np.int64)  # default -> zero row
        sel = m_e == m
        grid[flat_e[sel]] = plan["src_s"][sel]
        msgs = feat8x[grid.reshape(TOTBLK, P)]  # [TOTBLK, P, 128]
        msgs = np.ascontiguousarray(
            msgs.transpose(1, 0, 2).reshape(P, TOTBLK * P)
        )
        nz = nodes[m].reshape(-1)
        nidx = np.where(nz >= 0, nz, N)
        ftT = np.ascontiguousarray(featbx[nidx].T)
        rb = np.ascontiguousarray(recipx[nidx].reshape(1, NPC))
        in_maps.append(
            dict(
                msgs=msgs,
                featT=ftT,
                reciprow=rb,
                wsT=wsT,
                wnT=wnT,
                bias=bias,
                ident2=ident2,
            )
        )
    return in_maps


def _build(plan):
    NT = plan["NT"]
    NPC = plan["NPC"]
    KT = plan["KT"]
    BLK0 = plan["BLK0"]
    TOTBLK = plan["TOTBLK"]

    f32 = mybir.dt.float32
    bf16 = mybir.dt.bfloat16
    fp8 = mybir.dt.float8e4
    NS = NT // ST

    nc = bacc.Bacc(
        "TRN2",
        target_bir_lowering=False,
        debug=False,
        num_devices=NCORES,
    )

    msgs_t = nc.dram_tensor("msgs", [P, TOTBLK * P], fp8, kind="ExternalInput")
    ftT_t = nc.dram_tensor("featT", [P, NPC], bf16, kind="ExternalInput")
    rb_t = nc.dram_tensor("reciprow", [1, NPC], bf16, kind="ExternalInput")
    wsT_t = nc.dram_tensor("wsT", [P, P], bf16, kind="ExternalInput")
    wnT_t = nc.dram_tensor("wnT", [P, P], bf16, kind="ExternalInput")
    bias_t = nc.dram_tensor("bias", [P, 1], f32, kind="ExternalInput")
    id_t = nc.dram_tensor("ident2", [P, 2 * P], fp8, kind="ExternalInput")
    out_t = nc.dram_tensor("out", [P, NPC], bf16, kind="ExternalOutput")

    with tile.TileContext(nc) as tc:
        with (
            tc.tile_pool(name="const", bufs=1) as cpool,
            tc.tile_pool(name="hbuf", bufs=4 * ST) as hpool,
            tc.tile_pool(name="ps1", bufs=6, space="PSUM") as p1pool,
            tc.tile_pool(name="ps2", bufs=2, space="PSUM") as p2pool,
        ):
            id_sb = cpool.tile([P, 2 * P], fp8, tag="ident2")
            wsT_sb = cpool.tile([P, P], bf16, tag="ws")
            wnT_sb = cpool.tile([P, P], bf16, tag="wn")
            bias_sb = cpool.tile([P, 1], f32, tag="bias")
            rb_sb = cpool.tile([1, NPC], bf16, tag="rb")
            rbb_sb = cpool.tile([P, NPC], bf16, tag="rbb")
            ftT_sb = cpool.tile([P, NPC], bf16, tag="ftT")
            out_sb = cpool.tile([P, NPC], bf16, tag="out")
            msgs_sb = cpool.tile([P, TOTBLK * P], fp8, tag="msgs")

            # A priority-ordered DMA stream ping-ponged over the two HWDGE
            # queues (so one chunk's transfer overlaps the next chunk's
            # descriptor generation): the first (small) message chunk
            # unblocks TensorE ASAP; weights / recip / featT are slotted in
            # just before their first consumer would need them so they never
            # delay the message stream.
            qs = [nc.sync, nc.scalar]
            qi = [0]

            def issue(out_ap, in_ap):
                qs[qi[0] % len(qs)].dma_start(out_ap, in_ap)
                qi[0] += 1

            # message chunk boundaries: small taper-in (fast PE start), 48
            # blocks steady state, taper-out (small tail)
            head = [8, 16, 32]
            tail = [24, 16, 8]
            mid = TOTBLK - sum(head) - sum(tail)
            sizes = head + [48] * (mid // 48) + ([mid % 48] if mid % 48 else []) + tail
            bounds = np.concatenate([[0], np.cumsum(sizes)])
            assert bounds[-1] == TOTBLK

            # supertile end-blocks: rb slice s is first read by hb(s) (right
            # after agg(s)); ftT slice s is first read by stage2(s), which the
            # PE reaches right after agg(s+1) (pending depth 1)
            EB = [int(BLK0[(s + 1) * ST]) for s in range(NS)]
            done_s = 0
            W = ST * P
            for ci in range(len(bounds) - 1):
                b0, b1 = int(bounds[ci]), int(bounds[ci + 1])
                issue(msgs_sb[:, b0 * P : b1 * P], msgs_t.ap()[:, b0 * P : b1 * P])
                if ci == DELAY_CHUNKS:
                    issue(id_sb[:], id_t.ap()[:])
                while done_s < NS and EB[done_s] <= b1:
                    s = done_s
                    if s == 0:
                        issue(rb_sb[:], rb_t.ap()[:])
                        issue(wsT_sb[:], wsT_t.ap()[:])
                        issue(wnT_sb[:], wnT_t.ap()[:])
                        issue(bias_sb[:], bias_t.ap()[:])
                    if s >= 1:
                        f = s - 1
                        issue(
                            ftT_sb[:, f * W : (f + 1) * W],
                            ftT_t.ap()[:, f * W : (f + 1) * W],
                        )
                    done_s += 1
            for f in range(max(0, done_s - 1), NS):
                issue(
                    ftT_sb[:, f * W : (f + 1) * W], ftT_t.ap()[:, f * W : (f + 1) * W]
                )

            nc.gpsimd.partition_broadcast(rbb_sb[:], rb_sb[0:1, :])
            id3 = id_sb[:].rearrange("p (i e) -> p i e", e=P)
            msg3 = msgs_sb[:].rearrange("p (b e) -> p b e", e=P)
            DR = mybir.MatmulPerfMode.DoubleRow

            def emit_finish(fi):
                c0 = fi["s"] * ST * P
                ps2 = p2pool.tile([P, ST * P], f32, tag="ps2")
                for i, t in enumerate(fi["tiles"]):
                    sl = ps2[:, i * P : (i + 1) * P]
                    hb = fi["hbs"][i]
                    nc.tensor.matmul(
                        sl,
                        lhsT=wsT_sb[:],
                        rhs=ftT_sb[:, (c0 + i * P) : (c0 + (i + 1) * P)],
                        start=True,
                        stop=hb is None,
                    )
                    if hb is not None:
                        nc.tensor.matmul(
                            sl, lhsT=wnT_sb[:], rhs=hb[:], start=False, stop=True
                        )
                nc.scalar.activation(
                    out_sb[:, c0 : c0 + ST * P],
                    ps2[:],
                    mybir.ActivationFunctionType.Relu,
                    bias=bias_sb[:, 0:1],
                )
                issue(out_t.ap()[:, c0 : c0 + ST * P], out_sb[:, c0 : c0 + ST * P])

            pending = []
            for s in range(NS):
                tiles = list(range(s * ST, (s + 1) * ST))
                cb = int(BLK0[tiles[-1] + 1]) - int(BLK0[tiles[0]])
                hbs = []
                for i, t in enumerate(tiles):
                    k = int(KT[t])
                    if k == 0:
                        hbs.append(None)
                        continue
                    b0 = int(BLK0[t])
                    ps1t = p1pool.tile([P, P], f32, tag="ps1")
                    ps1 = ps1t[:]
                    npair = k // 2
                    for kk in range(npair):
                        nc.tensor.matmul(
                            ps1,
                            lhsT=msg3[:, b0 + 2 * kk : b0 + 2 * kk + 2, :],
                            rhs=id3,
                            start=(kk == 0),
                            stop=(kk == npair - 1 and k % 2 == 0),
                            perf_mode=DR,
                        )
                    if k % 2:
                        nc.tensor.matmul(
                            ps1,
                            lhsT=msg3[:, b0 + k - 1, :],
                            rhs=id3[:, 0, :],
                            start=(k == 1),
                            stop=True,
                        )
                    hb = hpool.tile([P, P], bf16, tag="hbuf")
                    nc.vector.tensor_tensor(
                        out=hb[:],
                        in0=ps1,
                        in1=rbb_sb[:, t * P : (t + 1) * P],
                        op=mybir.AluOpType.mult,
                    )
                    hbs.append(hb)
                fi = dict(s=s, tiles=tiles, hbs=hbs)
                pending.append(fi)
                if len(pending) > 1:
                    emit_finish(pending.pop(0))
            while pending:
                emit_finish(pending.pop(0))

    nc.compile()
    return nc


def kernel(feat, src, dst, W_self, b_self, W_neigh, b_neigh):
    feat = np.asarray(feat, np.float32)
    src = np.asarray(src, np.int64)
    dst = np.asarray(dst, np.int64)
    N, D = feat.shape
    assert D == P

    plan = _make_plan(src, dst, N)
    in_maps = _make_inputs(plan, feat, W_self, b_self, W_neigh, b_neigh)

    key = (N, D, plan["TOTBLK"], plan["KT"].tobytes())
    if LAST.get("key") != key:
        nc = _build(plan)
        LAST.update(key=key, nc=nc)
    nc = LAST["nc"]
    LAST["in_maps"] = in_maps

    res = run_bass_kernel_spmd(nc, in_maps, core_ids=list(range(NCORES)))
    out = np.zeros((N, P), np.float32)
    for m in range(NCORES):
        o = np.asarray(res.results[m]["out"]).astype(np.float32)  # [P, NPC]
        nz = plan["nodes"][m].reshape(-1)
        v = nz >= 0
        out[nz[v]] = o[:, v].T
    return out


# revision 26
# speedup vs baseline: 1.1117x; 1.0217x over previous
"""GraphSAGE-mean (DivFeatConv) forward on 8 TRN2 NeuronCores.

out = relu(feat @ W_self.T + b_self + segmean(feat[src], dst) @ W_neigh.T + b_neigh)

Strategy (SPMD, one program on 8 cores):
  - Nodes are relabeled by in-degree (descending) and dealt round-robin into
    128-node dst tiles so that every tile holds similar-degree nodes; tiles
    are dealt round-robin to cores so the per-tile block count K_j (a shared
    template constant = max over cores) is tight (~4% padding).
  - The host stages, per core, a "message image" in DRAM laid out exactly as
    the SBUF tile the kernel wants: partition p, block (B0[j]+k) holds the
    fp8 features of the k-th in-edge src of the j-th tile's p-th node (zero
    rows pad).  Slot POSITION encodes the dst node, so the scatter-sum is a
    matmul against a constant identity matrix and the "gather" is a plain
    sequential strided DMA - no SWDGE descriptor generation, no per-block
    selection matrices.
  - Aggregation: per dst tile, K_j blocks are summed in PSUM with fp8
    DoubleRow matmuls (two 128-slot blocks per instruction, 0.5 cyc/row):
    ps1[d, n] += msg_blk[slot, d]^T @ I[slot, n].
  - VectorE multiplies by 1/deg (bf16), TensorE applies W_self/W_neigh per
    512-column supertile, ScalarE fuses bias+ReLU, bf16 result DMAs out; the
    host casts to f32 and scatters rows back through the relabeling.
  - Message DMAs rotate over the sync/scalar/vector HWDGE queues and are
    sized ~65 blocks (~1MB) by greedy supertile balancing; stage 2 for
    supertile s is emitted after the aggregation matmuls of supertile s+1 so
    TensorE never waits on VectorE.

All template constants (K_j schedule) are maxima over cores, so one SPMD
program serves all 8 cores with per-core tables.
"""

import numpy as np
import ml_dtypes

import concourse.bacc as bacc
import concourse.bass as bass
import concourse.mybir as mybir
import concourse.tile as tile
from concourse.bass_utils import run_bass_kernel_spmd

BF16 = ml_dtypes.bfloat16
FP8 = ml_dtypes.float8_e4m3
P = 128
NCORES = 8
ST = 4  # dst tiles per supertile (ps2 = [128, ST*128] f32 = one PSUM bank)
DELAY_CHUNKS = 0  # message chunks landed before the PE is released

# stash of the last compiled/run state so test harnesses can re-run with
# tracing enabled
LAST = {}


def _make_plan(src, dst, n_nodes):
    """Host-side relabeling / edge partitioning. Returns the shared template
    plus per-core node/edge layout."""
    N = n_nodes
    E = len(dst)
    deg = np.bincount(dst, minlength=N)
    order = np.argsort(-deg, kind="stable").astype(np.int64)
    NTG = -(-N // P)          # global 128-node tiles
    NT = -(-NTG // NCORES)    # local tiles per core
    NPC = NT * P              # output columns per core

    # core m local tile j <- global tile NCORES*j + m (degree-sorted)
    nodes = np.full((NCORES, NT, P), -1, np.int64)
    for j in range(NT):
        for m in range(NCORES):
            g = NCORES * j + m
            if g >= NTG:
                continue
            ids = order[P * g : P * g + P]
            nodes[m, j, : len(ids)] = ids

    degw = np.where(nodes >= 0, deg[np.clip(nodes, 0, None)], 0)
    KT = degw.max(axis=2).max(axis=0)  # [NT] blocks per tile (template)

    # greedy-balance tiles into NT/ST supertiles of exactly ST tiles each,
    # then flatten back into a processing order; columns/blocks follow the
    # processing order so DMA chunks and ps2 groups stay contiguous.
    NS = NT // ST
    bins = [[] for _ in range(NS)]
    loads = np.zeros(NS, np.int64)
    for t in np.argsort(-KT, kind="stable"):
        open_bins = [b for b in range(NS) if len(bins[b]) < ST]
        b = min(open_bins, key=lambda x: (loads[x], x))
        bins[b].append(int(t))
        loads[b] += KT[t]
    proc = np.array([t for b in bins for t in sorted(b)], np.int64)

    nodes = nodes[:, proc, :]          # [NC, NT, P] in processing order
    KTp = KT[proc]
    BLK0 = np.zeros(NT + 1, np.int64)
    np.cumsum(KTp, out=BLK0[1:])
    TOTBLK = int(BLK0[-1])

    # per-node placement maps (for vectorized edge binning)
    node_core = np.full(N, -1, np.int64)
    node_col = np.zeros(N, np.int64)    # column within core [0, NPC)
    node_blk0 = np.zeros(N, np.int64)   # block offset of its tile
    cols = np.arange(NT * P, dtype=np.int64)
    blk0_of_col = np.repeat(BLK0[:-1], P)
    for m in range(NCORES):
        nz = nodes[m].reshape(-1)
        v = nz >= 0
        node_core[nz[v]] = m
        node_col[nz[v]] = cols[v]
        node_blk0[nz[v]] = blk0_of_col[v]

    # CSR by dst
    eorder = np.argsort(dst, kind="stable")
    dst_s = dst[eorder]
    src_s = src[eorder]
    indptr = np.zeros(N + 1, np.int64)
    np.cumsum(deg, out=indptr[1:])
    k_e = np.arange(E, dtype=np.int64) - indptr[dst_s]  # rank within dst

    recip = (1.0 / np.maximum(deg, 1)).astype(np.float32)

    return dict(
        N=N,
        NT=NT,
        NPC=NPC,
        KT=KTp,
        BLK0=BLK0,
        TOTBLK=TOTBLK,
        nodes=nodes,
        node_core=node_core,
        node_col=node_col,
        node_blk0=node_blk0,
        dst_s=dst_s,
        src_s=src_s,
        k_e=k_e,
        recip=recip,
    )


def _make_inputs(plan, feat, W_self, b_self, W_neigh, b_neigh):
    N = plan["N"]
    NPC = plan["NPC"]
    TOTBLK = plan["TOTBLK"]
    nodes = plan["nodes"]

    feat8x = np.vstack([feat.astype(FP8), np.zeros((1, P), FP8)])
    featbx = np.vstack([feat.astype(BF16), np.zeros((1, P), BF16)])
    recipx = np.append(plan["recip"], 0.0).astype(BF16)

    wsT = np.ascontiguousarray(np.asarray(W_self, np.float32).T).astype(BF16)
    wnT = np.ascontiguousarray(np.asarray(W_neigh, np.float32).T).astype(BF16)
    bias = (
        (np.asarray(b_self, np.float32) + np.asarray(b_neigh, np.float32))
        .astype(np.float32)
        .reshape(P, 1)
    )
    ident2 = np.zeros((P, 2 * P), FP8)
    ident2[np.arange(P), np.arange(P)] = 1.0
    ident2[np.arange(P), P + np.arange(P)] = 1.0

    m_e = plan["node_core"][plan["dst_s"]]
    flat_e = (plan["node_blk0"][plan["dst_s"]] + plan["k_e"]) * P + (
        plan["node_col"][plan["dst_s"]] % P
    )

    in_maps = []
    for m in range(NCORES):
        grid = np.full(TOTBLK * P, N, np.int64)  # default -> zero row
        sel = m_e == m
        grid[flat_e[sel]] = plan["src_s"][sel]
        msgs = feat8x[grid.reshape(TOTBLK, P)]  # [TOTBLK, P, 128]
        msgs = np.ascontiguousarray(
            msgs.transpose(1, 0, 2).reshape(P, TOTBLK * P)
        )
        nz = nodes[m].reshape(-1)
        nidx = np.where(nz >= 0, nz, N)
        ftT = np.ascontiguousarray(featbx[nidx].T)
        rb = np.ascontiguousarray(recipx[nidx].reshape(1, NPC))
        in_maps.append(
            dict(
                msgs=msgs,
                featT=ftT,
                reciprow=rb,
                wsT=wsT,
                wnT=wnT,
                bias=bias,
                ident2=ident2,
            )
        )
    return in_maps


def _build(plan):
    NT = plan["NT"]
    NPC = plan["NPC"]
    KT = plan["KT"]
    BLK0 = plan["BLK0"]
    TOTBLK = plan["TOTBLK"]

    f32 = mybir.dt.float32
    bf16 = mybir.dt.bfloat16
    fp8 = mybir.dt.float8e4
    NS = NT // ST

    nc = bacc.Bacc(
        "TRN2",
        target_bir_lowering=False,
        debug=False,
        num_devices=NCORES,
    )

    msgs_t = nc.dram_tensor("msgs", [P, TOTBLK * P], fp8, kind="ExternalInput")
    ftT_t = nc.dram_tensor("featT", [P, NPC], bf16, kind="ExternalInput")
    rb_t = nc.dram_tensor("reciprow", [1, NPC], bf16, kind="ExternalInput")
    wsT_t = nc.dram_tensor("wsT", [P, P], bf16, kind="ExternalInput")
    wnT_t = nc.dram_tensor("wnT", [P, P], bf16, kind="ExternalInput")
    bias_t = nc.dram_tensor("bias", [P, 1], f32, kind="ExternalInput")
    id_t = nc.dram_tensor("ident2", [P, 2 * P], fp8, kind="ExternalInput")
    out_t = nc.dram_tensor("out", [P, NPC], bf16, kind="ExternalOutput")

    with tile.TileContext(nc) as tc:
        with (
            tc.tile_pool(name="const", bufs=1) as cpool,
            tc.tile_pool(name="hbuf", bufs=4 * ST) as hpool,
            tc.tile_pool(name="ps1", bufs=6, space="PSUM") as p1pool,
            tc.tile_pool(name="ps2", bufs=2, space="PSUM") as p2pool,
        ):
            id_sb = cpool.tile([P, 2 * P], fp8, tag="ident2")
            wsT_sb = cpool.tile([P, P], bf16, tag="ws")
            wnT_sb = cpool.tile([P, P], bf16, tag="wn")
            bias_sb = cpool.tile([P, 1], f32, tag="bias")
            rb_sb = cpool.tile([1, NPC], bf16, tag="rb")
            rbb_sb = cpool.tile([P, NPC], bf16, tag="rbb")
            ftT_sb = cpool.tile([P, NPC], bf16, tag="ftT")
            out_sb = cpool.tile([P, NPC], bf16, tag="out")
            msgs_sb = cpool.tile([P, TOTBLK * P], fp8, tag="msgs")

            # A priority-ordered DMA stream ping-ponged over the two HWDGE
            # queues (so one chunk's transfer overlaps the next chunk's
            # descriptor generation): the first (small) message chunk
            # unblocks TensorE ASAP; weights / recip / featT are slotted in
            # just before their first consumer would need them so they never
            # delay the message stream.
            qs = [nc.sync, nc.scalar]
            qi = [0]

            def issue(out_ap, in_ap):
                qs[qi[0] % len(qs)].dma_start(out_ap, in_ap)
                qi[0] += 1

            # message chunk boundaries: small taper-in (fast PE start), 48
            # blocks steady state, taper-out (small tail)
            head = [8, 16, 32]
            tail = [24, 16, 8]
            mid = TOTBLK - sum(head) - sum(tail)
            sizes = head + [48] * (mid // 48) + ([mid % 48] if mid % 48 else []) + tail
            bounds = np.concatenate([[0], np.cumsum(sizes)])
            assert bounds[-1] == TOTBLK

            # supertile end-blocks: rb slice s is first read by hb(s) (right
            # after agg(s)); ftT slice s is first read by stage2(s), which the
            # PE reaches right after agg(s+1) (pending depth 1)
            EB = [int(BLK0[(s + 1) * ST]) for s in range(NS)]
            done_s = 0
            W = ST * P
            issue(rb_sb[:], rb_t.ap()[:])
            for s in range(NS):
                nc.gpsimd.partition_broadcast(
                    rbb_sb[:, s * W : (s + 1) * W], rb_sb[0:1, s * W : (s + 1) * W]
                )
            for ci in range(len(bounds) - 1):
                b0, b1 = int(bounds[ci]), int(bounds[ci + 1])
                issue(msgs_sb[:, b0 * P : b1 * P], msgs_t.ap()[:, b0 * P : b1 * P])
                if ci == DELAY_CHUNKS:
                    issue(id_sb[:], id_t.ap()[:])
                while done_s < NS and EB[done_s] <= b1:
                    s = done_s
                    if s == 0:
                        issue(wsT_sb[:], wsT_t.ap()[:])
                        issue(wnT_sb[:], wnT_t.ap()[:])
                        issue(bias_sb[:], bias_t.ap()[:])
                    if s >= 1:
                        f = s - 1
                        issue(
                            ftT_sb[:, f * W : (f + 1) * W],
                            ftT_t.ap()[:, f * W : (f + 1) * W],
                        )
                    done_s += 1
            for f in range(max(0, done_s - 1), NS):
                issue(
                    ftT_sb[:, f * W : (f + 1) * W], ftT_t.ap()[:, f * W : (f + 1) * W]
                )

            id3 = id_sb[:].rearrange("p (i e) -> p i e", e=P)
            msg3 = msgs_sb[:].rearrange("p (b e) -> p b e", e=P)
            DR = mybir.MatmulPerfMode.DoubleRow

            def emit_finish(fi):
                c0 = fi["s"] * ST * P
                ps2 = p2pool.tile([P, ST * P], f32, tag="ps2")
                for i, t in enumerate(fi["tiles"]):
                    sl = ps2[:, i * P : (i + 1) * P]
                    hb = fi["hbs"][i]
                    nc.tensor.matmul(
                        sl,
                        lhsT=wsT_sb[:],
                        rhs=ftT_sb[:, (c0 + i * P) : (c0 + (i + 1) * P)],
                        start=True,
                        stop=hb is None,
                    )
                    if hb is not None:
                        nc.tensor.matmul(
                            sl, lhsT=wnT_sb[:], rhs=hb[:], start=False, stop=True
                        )
                nc.scalar.activation(
                    out_sb[:, c0 : c0 + ST * P],
                    ps2[:],
                    mybir.ActivationFunctionType.Relu,
                    bias=bias_sb[:, 0:1],
                )
                issue(out_t.ap()[:, c0 : c0 + ST * P], out_sb[:, c0 : c0 + ST * P])

            pending = []
            for s in range(NS):
                tiles = list(range(s * ST, (s + 1) * ST))
                cb = int(BLK0[tiles[-1] + 1]) - int(BLK0[tiles[0]])
                hbs = []
                for i, t in enumerate(tiles):
                    k = int(KT[t])
                    if k == 0:
                        hbs.append(None)
                        continue
                    b0 = int(BLK0[t])
                    ps1t = p1pool.tile([P, P], f32, tag="ps1")
                    ps1 = ps1t[:]
                    npair = k // 2
                    for kk in range(npair):
                        nc.tensor.matmul(
                            ps1,
                            lhsT=msg3[:, b0 + 2 * kk : b0 + 2 * kk + 2, :],
                            rhs=id3,
                            start=(kk == 0),
                            stop=(kk == npair - 1 and k % 2 == 0),
                            perf_mode=DR,
                        )
                    if k % 2:
                        nc.tensor.matmul(
                            ps1,
                            lhsT=msg3[:, b0 + k - 1, :],
                            rhs=id3[:, 0, :],
                            start=(k == 1),
                            stop=True,
                        )
                    hb = hpool.tile([P, P], bf16, tag="hbuf")
                    nc.vector.tensor_tensor(
                        out=hb[:],
                        in0=ps1,
                        in1=rbb_sb[:, t * P : (t + 1) * P],
                        op=mybir.AluOpType.mult,
                    )
                    hbs.append(hb)
                fi = dict(s=s, tiles=tiles, hbs=hbs)
                pending.append(fi)
                if len(pending) > 1:
                    emit_finish(pending.pop(0))
            while pending:
                emit_finish(pending.pop(0))

    nc.compile()
    return nc


def kernel(feat, src, dst, W_self, b_self, W_neigh, b_neigh):
    feat = np.asarray(feat, np.float32)
    src = np.asarray(src, np.int64)
    dst = np.asarray(dst, np.int64)
    N, D = feat.shape
    assert D == P

    plan = _make_plan(src, dst, N)
    in_maps = _make_inputs(plan, feat, W_self, b_self, W_neigh, b_neigh)

    key = (N, D, plan["TOTBLK"], plan["KT"].tobytes())
    if LAST.get("key") != key:
        nc = _build(plan)
        LAST.update(key=key, nc=nc)
    nc = LAST["nc"]
    LAST["in_maps"] = in_maps

    res = run_bass_kernel_spmd(nc, in_maps, core_ids=list(range(NCORES)))
    out = np.zeros((N, P), np.float32)
    for m in range(NCORES):
        o = np.asarray(res.results[m]["out"]).astype(np.float32)  # [P, NPC]
        nz = plan["nodes"][m].reshape(-1)
        v = nz >= 0
        out[nz[v]] = o[:, v].T
    return out
